# revision 2
# baseline (speedup 1.0000x reference)
"""MultiHeadAttention Trainium2 kernel, v2.

Full inputs -> full output. Sharding: 8 cores = (batch b in 0..3) x (head
group g in 0..1, 8 heads each). Core projects Q/K/V for its head group,
runs attention for its 8 heads, applies its half of the output projection,
returns a partial [2048, 1024] f32 output summed on host (+ bias).

v2 design (vs baseline): the kernel is paced by the scalar engine's exp
stream (256 ACTs x ~1.2us = the hard floor), so everything else hides
under it:
  - minimal prologue (K c0 + Q c0 blocks 0-1) -> first exp at ~20us
    instead of ~158us; remaining projections/V/out-proj run as PE filler
    work between logits matmuls.
  - logits for the even/odd head of a pair issue back-to-back as 64x128
    array row-tiles T0/T8 (auto tile_position from base partitions) ->
    both stream concurrently, ~2x logits throughput.
  - softmax denominator: va holds [v_even(64) | em(64) | v_odd(64)] per
    (key-chunk, pair); the shared em block replicates the masked-ones
    column 64x, so the ctx matmul (M=128) lands the denominator
    replicated on the opposite partition half. DVE reciprocal there,
    SBUF->SBUF DMA moves it onto the v lanes, one DVE mul normalizes.
    No PE broadcast/shift matmuls, no PSUM bank for normalization.
  - sqb-major loop (2 blocks of 1024 queries); ctx for stretch k runs
    as bursts during stretch k+1; out-proj per 128-row chunk interleaves
    so the tail is small.
"""

import numpy as np
import ml_dtypes
from collections import deque

import concourse.bass as bass
import concourse.mybir as mybir
import concourse.tile as tile
from concourse import bacc
from concourse.bass_utils import run_bass_kernel_spmd

f32 = mybir.dt.float32
bf16 = mybir.dt.bfloat16
np_bf16 = ml_dtypes.bfloat16

B, S, D, H, DH = 4, 2048, 1024, 16, 64
HG = H // 2          # 8 heads per core
DG = HG * DH         # 512 projection cols per core
N_CORES = 8
Exp = mybir.ActivationFunctionType.Exp

KC = D // 128        # 8 contraction chunks over d_model
CC = 4               # head pairs per core
SKC = S // 128       # 16 key chunks
SB = 2               # query super-blocks of 1024
QB = 1024            # queries per super-block


def _build():
    nc = bacc.Bacc(None, target_bir_lowering=False)

    xq = nc.dram_tensor("xq", [D, S], bf16, kind="ExternalInput")
    xk = nc.dram_tensor("xk", [D, S], bf16, kind="ExternalInput")
    xv = nc.dram_tensor("xv", [D, S], bf16, kind="ExternalInput")
    wq = nc.dram_tensor("wq", [D, DG], bf16, kind="ExternalInput")
    wk = nc.dram_tensor("wk", [D, DG], bf16, kind="ExternalInput")
    wv = nc.dram_tensor("wv", [D, DG], bf16, kind="ExternalInput")
    wo = nc.dram_tensor("wo", [DG, D], bf16, kind="ExternalInput")
    bqk = nc.dram_tensor("bqk", [128, 8], f32, kind="ExternalInput")
    emf = nc.dram_tensor("emf", [128, SKC], f32, kind="ExternalInput")
    ones2 = nc.dram_tensor("ones2", [128, 256], bf16, kind="ExternalInput")
    out = nc.dram_tensor("out", [S, D], f32, kind="ExternalOutput")

    with tile.TileContext(nc) as tc:
        _emit(nc, tc, xq, xk, xv, wq, wk, wv, wo, bqk, emf, ones2, out)
    nc.finalize()
    return nc


def _emit(nc, tc, xq, xk, xv, wq, wk, wv, wo, bqk, emf, ones2, out):
    from contextlib import ExitStack

    with ExitStack() as ctx:
        persist = ctx.enter_context(tc.tile_pool(name="persist", bufs=1))
        wpool = ctx.enter_context(tc.tile_pool(name="wpool", bufs=3))
        wopool = ctx.enter_context(tc.tile_pool(name="wopool", bufs=1))
        xpool = ctx.enter_context(tc.tile_pool(name="xpool", bufs=2))
        ptp = ctx.enter_context(tc.tile_pool(name="ptp", bufs=22))
        recp = ctx.enter_context(tc.tile_pool(name="recp", bufs=5))
        stg = ctx.enter_context(tc.tile_pool(name="stg", bufs=2))
        psl = ctx.enter_context(tc.tile_pool(name="psl", bufs=2, space="PSUM"))
        psc = ctx.enter_context(tc.tile_pool(name="psc", bufs=2, space="PSUM"))
        pso = ctx.enter_context(tc.tile_pool(name="pso", bufs=2, space="PSUM"))

        # ---------- persistent SBUF ----------
        qt_sb = persist.tile([128, CC, S], bf16, name="qt")   # head h: partitions (h%2)*64, chunk h//2
        kt_sb = persist.tile([128, CC, S], bf16, name="kt")
        cx_sb = persist.tile([128, CC, S], bf16, name="cx")
        # per (sc, pair): [v_even(64) | em(64) | v_odd(64)]
        va_sb = persist.tile([128, SKC, CC * 192], bf16, name="va")
        bias_sb = persist.tile([128, 8], f32, name="bias")
        emf_sb = persist.tile([128, SKC], f32, name="emf")
        ones_sb = persist.tile([128, 256], bf16, name="ones")
        dummy_sb = persist.tile([128, 256], bf16, name="dummy")

        # ---------- input DMAs (priority order) ----------
        nc.gpsimd.memset(dummy_sb[:], 0.25)
        nc.sync.dma_start(bias_sb, bqk[:])
        nc.sync.dma_start(emf_sb, emf[:])
        nc.sync.dma_start(ones_sb, ones2[:])
        wk_t = wpool.tile([128, KC, DG], bf16, tag="w", name="wk_t")
        nc.sync.dma_start(wk_t, wk[:].rearrange("(ko p) c -> p ko c", p=128))
        wq_t = wpool.tile([128, KC, DG], bf16, tag="w", name="wq_t")
        nc.sync.dma_start(wq_t, wq[:].rearrange("(ko p) c -> p ko c", p=128))
        xk_t = {}
        xq_t = {}
        xk_t[(0, 0)] = xpool.tile([128, KC, 512], bf16, tag="xk", name="xk_t")
        nc.sync.dma_start(xk_t[(0, 0)],
                          xk[:, 0:512].rearrange("(ko p) s -> p ko s", p=128))

        # PE warmup spam while DMAs land (HAM un-throttle) + ACT table
        # preload (first Exp pays ~2.7us table DMA otherwise)
        warm_act = persist.tile([128, 256], bf16, name="warm_act")
        nc.scalar.activation(warm_act, dummy_sb[:], Exp, scale=0.125)
        for i in range(20):
            pw = pso.tile([128, 512], f32, tag="pso", name="warm")
            nc.tensor.matmul(pw[:, 0:256], lhsT=dummy_sb[:, 0:128],
                             rhs=dummy_sb[:, 0:256], start=True, stop=True)

        # em fill: va[:, sc, pair*192+64 : +128] = ones * emf[:, sc]
        def em_fill(sc):
            dst = va_sb[:, sc, :].rearrange("p (c x) -> p c x", c=CC)[:, :, 64:128]
            src = ones_sb[:].rearrange("p (c x) -> p c x", c=CC)
            with nc.allow_low_precision(reason="em in bf16"):
                nc.vector.tensor_scalar_mul(dst, src, emf_sb[:, sc:sc + 1])

        # ---------- work item emitters ----------
        def xk_dma(key):
            """Load xk block key[1] into the ring slot keyed (stretch, blk).

            Blocks 1-3 are re-DMA'd per stretch so ring-buffer waits never
            cross a stretch boundary (cross-stretch sync-queue blocking
            deadlocks against the norm-path rec DMAs).
            """
            blk = key[1]
            xk_t[key] = xpool.tile([128, KC, 512], bf16, tag="xk", name="xk_t")
            nc.sync.dma_start(xk_t[key], xk[:, blk * 512:(blk + 1) * 512]
                              .rearrange("(ko p) s -> p ko s", p=128))

        def proj_item(which, cc, blk, xkey=None):
            """Project one (cc, blk) chunk of Q or K into qt/kt."""
            w_t, x_t, dst, bcol = {
                "q": (wq_t, xq_t, qt_sb, 0),
                "k": (wk_t, xk_t, kt_sb, 4),
            }[which]
            xt = x_t[xkey if xkey is not None else blk]
            ps = pso.tile([128, 512], f32, tag="pso", name="proj")
            for kc in range(KC):
                nc.tensor.matmul(ps, lhsT=w_t[:, kc, cc * 128:(cc + 1) * 128],
                                 rhs=xt[:, kc, :],
                                 start=(kc == 0), stop=(kc == KC - 1))
            with nc.allow_low_precision(reason="proj rounded to bf16"):
                nc.vector.tensor_scalar_add(
                    dst[:, cc, blk * 512:(blk + 1) * 512], ps,
                    bias_sb[:, bcol + cc:bcol + cc + 1])

        wv_t = [None]
        xv_t = {}

        def v_dma(sc2):
            """DMA xv for a pair of key chunks (256 cols)."""
            xv_t[sc2] = xpool.tile([128, KC, 256], bf16, tag="xv", name="xv_t")
            nc.sync.dma_start(xv_t[sc2], xv[:, sc2 * 256:(sc2 + 1) * 256]
                              .rearrange("(ko p) s -> p ko s", p=128))

        def v_item(sc):
            """Project V for key chunk sc and scatter into va (scaled by em)."""
            ps = pso.tile([128, 512], f32, tag="pso", name="vproj")
            xvt = xv_t[sc // 2]
            off = (sc % 2) * 128
            for kc in range(KC):
                nc.tensor.matmul(ps, lhsT=xvt[:, kc, off:off + 128],
                                 rhs=wv_t[0][:, kc, :],
                                 start=(kc == 0), stop=(kc == KC - 1))
            psv = ps.rearrange("p (c par x) -> p c par x", c=CC, par=2)
            dst = va_sb[:, sc, :].rearrange("p (c x) -> p c x", c=CC)
            with nc.allow_low_precision(reason="va in bf16"):
                nc.vector.tensor_scalar_mul(dst[:, :, 0:64], psv[:, :, 0, :],
                                            emf_sb[:, sc:sc + 1])
                nc.vector.tensor_scalar_mul(dst[:, :, 128:192], psv[:, :, 1, :],
                                            emf_sb[:, sc:sc + 1])

        # ---------- logits / exp slots ----------
        pt_slots = {}

        def logits_slot_pair(s, p, j):
            """Both head parities of pair p, key chunk j: 4 N=512 matmuls
            fully alternating T0/T8 row-tiles so they stream concurrently."""
            pls = []
            for par in range(2):
                pls.append(psl.tile([128, QB], f32, tag="psl", name="psl"))
            for h2 in range(2):
                for par in range(2):
                    lo = 64 * par
                    nc.tensor.matmul(
                        pls[par][:, h2 * 512:(h2 + 1) * 512],
                        lhsT=kt_sb[lo:lo + 64, p, j * 128:(j + 1) * 128],
                        rhs=qt_sb[lo:lo + 64, p, s * QB + h2 * 512:s * QB + h2 * 512 + 512],
                        start=True, stop=True)
            for par in range(2):
                pt = ptp.tile([128, QB], bf16, tag="pt", name="pt")
                nc.scalar.activation(pt, pls[par], Exp, scale=0.125)
                pt_slots[(s, p, j, par)] = pt

        # ---------- ctx + normalization ----------
        def ctx_item(s, p, par, accs, j0, nj):
            """nj key chunks of ctx accumulation for (s, p, parity)."""
            base = p * 192 + (64 if par else 0)
            for j in range(j0, j0 + nj):
                pt = pt_slots[(s, p, j, par)]
                for half in range(2):
                    nc.tensor.matmul(accs[half],
                                     lhsT=va_sb[:, j, base:base + 128],
                                     rhs=pt[:, half * 512:(half + 1) * 512],
                                     start=(j == 0), stop=(j == SKC - 1))
                del pt_slots[(s, p, j, par)]

        def norm_item(s, p, par, accs):
            """Reciprocal of the replicated denominator + move + normalize.

            reciprocal_approx_fast only works on partitions 0-63 (silent
            no-op on 64-127), so route the denominator there first.
            """
            hcol = s * QB
            for half in range(2):
                acc = accs[half]
                rv = recp.tile([128, 512], f32, tag="rec", name="recv")
                if par == 0:
                    # den rows 64-127, v rows 0-63: copy den down, rcp low
                    dsb = recp.tile([128, 512], f32, tag="rec", name="dsb")
                    nc.vector.tensor_copy(dsb[64:128, :], acc[64:128, :])
                    d2 = recp.tile([128, 512], f32, tag="rec", name="d2")
                    nc.sync.dma_start(d2[0:64, :], dsb[64:128, :])
                    nc.vector.reciprocal_approx_fast(rv[0:64, :], d2[0:64, :])
                    vlo = 0
                else:
                    # den rows 0-63, v rows 64-127: rcp low, DMA up
                    rd = recp.tile([128, 512], f32, tag="rec", name="recd")
                    nc.vector.reciprocal_approx_fast(rd[0:64, :], acc[0:64, :])
                    nc.sync.dma_start(rv[64:128, :], rd[0:64, :])
                    vlo = 64
                dst = cx_sb[vlo:vlo + 64, p,
                            hcol + half * 512:hcol + half * 512 + 512]
                with nc.allow_low_precision(reason="ctxn in bf16"):
                    nc.vector.tensor_mul(out=dst, in0=acc[vlo:vlo + 64, :],
                                         in1=rv[vlo:vlo + 64, :])

        # ---------- output projection ----------
        wo_t = [None]

        def outproj_item(s, st8):
            row = (s * 8 + st8) * 128
            ot = stg.tile([128, 1024], f32, tag="stg", name="ot")
            for half in range(2):
                ps = pso.tile([128, 512], f32, tag="pso", name="ops")
                for cc in range(CC):
                    nc.tensor.matmul(ps,
                                     lhsT=cx_sb[:, cc, row:row + 128],
                                     rhs=wo_t[0][:, cc, half * 512:(half + 1) * 512],
                                     start=(cc == 0), stop=(cc == CC - 1))
                nc.vector.tensor_copy(ot[:, half * 512:(half + 1) * 512], ps)
            nc.sync.dma_start(out[row:row + 128, :], ot)

        # ---------- schedule ----------
        stretches = [(s, p) for s in range(SB) for p in range(CC)]
        NSTRETCH = len(stretches)

        def wv_dma():
            wv_t[0] = wpool.tile([128, KC, DG], bf16, tag="w", name="wv_t")
            nc.sync.dma_start(wv_t[0], wv[:].rearrange("(ko p) c -> p ko c", p=128))

        def wo_dma():
            wo_t[0] = wopool.tile([128, CC, D], bf16, tag="wo", name="wo_t")
            nc.sync.dma_start(wo_t[0], wo[:].rearrange("(co p) c -> p co c", p=128))

        def xq_dma(blk):
            xq_t[blk] = xpool.tile([128, KC, 512], bf16, tag="xq", name="xq_t")
            nc.sync.dma_start(xq_t[blk], xq[:, blk * 512:(blk + 1) * 512]
                              .rearrange("(ko p) s -> p ko s", p=128))

        # prologue: em fills + minimal projections for stretch-0 slot 0
        for sc in range(SKC):
            em_fill(sc)
        xq_dma(0)
        xq_dma(1)
        proj_item("k", 0, 0, xkey=(0, 0))
        proj_item("q", 0, 0)
        proj_item("q", 0, 1)
        wv_dma()

        def ctx_norm_items(k, par):
            """Filler items consuming stretch k's pt slots for one parity."""
            s, p = stretches[k]
            items = []
            accs = {}

            def mk_start(accs=accs):
                accs[0] = psc.tile([128, 512], f32, tag="psc", name="psc")
                accs[1] = psc.tile([128, 512], f32, tag="psc", name="psc")

            items.append((10, None, mk_start))
            for j0 in range(0, SKC, 2):
                items.append((1000, None,
                              lambda s=s, p=p, par=par, accs=accs, j0=j0:
                              ctx_item(s, p, par, accs, j0, 2)))
            items.append((900, None, lambda s=s, p=p, par=par, accs=accs:
                          norm_item(s, p, par, accs)))
            return items

        # item = (cost_ns, deadline (k, j) or None, fn); deque order is a
        # valid topological order; deadlines force-pop before a logits slot
        # that depends on the item.
        work = deque()
        plan = {k: [] for k in range(NSTRETCH + 1)}

        def K(c, b, dl, xkey=None):
            return (2300, dl, lambda: proj_item("k", c, b, xkey=xkey))

        def XK(key):
            return (10, None, lambda: xk_dma(key))

        def Q(c, b, dl):
            return (2300, dl, lambda: proj_item("q", c, b))

        def V(sc):
            return (2300, (1, sc), lambda: v_item(sc))

        # stretch 0: remaining K block-0 chunks + c0's later blocks paced by
        # deadline; other chunks' later blocks re-DMA'd in their stretches.
        plan[0] += [XK((0, 1)), XK((0, 2)),
                    K(1, 0, (0, 1), (0, 0)), K(2, 0, (0, 2), (0, 0)),
                    K(3, 0, (0, 3), (0, 0)),
                    K(0, 1, (0, 4), (0, 1)),
                    XK((0, 3)),
                    (10, None, lambda: v_dma(0)),
                    K(0, 2, (0, 8), (0, 2)),
                    (10, None, lambda: v_dma(1)),
                    K(0, 3, (0, 12), (0, 3)),
                    Q(1, 0, (0, 13)), Q(1, 1, (0, 14)),
                    (10, None, lambda: v_dma(2)),
                    V(0), V(1)]
        plan[1] += [XK((1, 1)), XK((1, 2)), K(1, 1, (1, 4), (1, 1)),
                    XK((1, 3)), K(1, 2, (1, 8), (1, 2)),
                    K(1, 3, (1, 12), (1, 3))]
        plan[2] = [(10, None, wo_dma), Q(3, 0, (3, 0)), Q(3, 1, (3, 0)),
                   XK((2, 1)), XK((2, 2)), K(2, 1, (2, 4), (2, 1)),
                   XK((2, 3)), K(2, 2, (2, 8), (2, 2)),
                   K(2, 3, (2, 12), (2, 3))]
        plan[3] = [(10, None, lambda: xq_dma(2)), (10, None, lambda: xq_dma(3)),
                   Q(0, 2, (4, 0)), Q(0, 3, (4, 0)),
                   XK((3, 1)), XK((3, 2)), K(3, 1, (3, 4), (3, 1)),
                   XK((3, 3)), K(3, 2, (3, 8), (3, 2)),
                   K(3, 3, (3, 12), (3, 3))]
        plan[4] = [Q(1, 2, (5, 0)), Q(1, 3, (5, 0)),
                   Q(2, 2, (6, 0)), Q(2, 3, (6, 0))]
        plan[5] = [Q(3, 2, (7, 0)), Q(3, 3, (7, 0))]
        plan[5] += [(2500, None, lambda c=c: outproj_item(0, c)) for c in (0, 1, 2)]
        plan[6] = [(2500, None, lambda c=c: outproj_item(0, c)) for c in (3, 4, 5, 6, 7)]
        plan[7] = []
        plan[8] = [(2500, None, lambda c=c: outproj_item(1, c)) for c in range(8)]

        for k, (s, p) in enumerate(stretches):
            if k == 1:
                # interleave remaining V items ahead of the ctx(s0) bursts
                ce = ctx_norm_items(0, 0)
                co = ctx_norm_items(0, 1)
                # ce = [mk, c0, c2, ..., c14, norm]; ctx item j0 needs va
                # up to sc j0+1, so V(sc) precedes the item that uses it.
                def Dv(s2):
                    return (10, None, lambda: v_dma(s2))

                merged = [V(2), V(3), Dv(3), V(4), V(5), Dv(4)] + ce[0:4] \
                    + [V(6), V(7), Dv(5), V(8), V(9)] + ce[4:6] \
                    + [Dv(6), V(10), V(11), V(12), V(13)] + ce[6:8] \
                    + [Dv(7), V(14), V(15)] + ce[8:] \
                    + [Q(2, 0, (2, 0)), Q(2, 1, (2, 0))] + co
                work.extend(merged)
            elif k > 1:
                for par in range(2):
                    work.extend(ctx_norm_items(k - 1, par))
            work.extend(plan[k])
            last = k == NSTRETCH - 1
            accs_e = {}
            for j in range(SKC):
                # force-pop through the last item whose deadline has arrived
                # (deque order is topological, so everything ahead of it
                # must run too)
                need = -1
                for idx, it in enumerate(work):
                    if it[1] is not None and it[1] <= (k, j):
                        need = idx
                for _ in range(need + 1):
                    _, _, fn = work.popleft()
                    fn()
                logits_slot_pair(s, p, j)
                if last:
                    # even-parity ctx of the final stretch runs inline,
                    # accumulating in the (now idle) pso pool
                    if j == 1:
                        accs_e[0] = pso.tile([128, 512], f32, tag="pso",
                                             name="psce")
                        accs_e[1] = pso.tile([128, 512], f32, tag="pso",
                                             name="psce")
                    if j >= 1:
                        ctx_item(s, p, 0, accs_e, j - 1, 1)
                # lighter filler load in the first slots of a stretch: the
                # ctx-burst handoff + force-pops otherwise starve the ACT
                budget = (2200 if k < 3 else 1700) if j >= 3 else 1100
                while work and budget > 0:
                    cost, _, fn = work.popleft()
                    fn()
                    budget -= cost

        # epilogue: finish final stretch's ctx (E inline leftover + O burst),
        # then the s1 output projection
        s, p = stretches[-1]
        ctx_item(s, p, 0, accs_e, SKC - 1, 1)
        work.append((900, None, lambda: norm_item(s, p, 0, accs_e)))
        work.extend(ctx_norm_items(NSTRETCH - 1, 1))
        work.extend(plan[NSTRETCH])
        while work:
            _, _, fn = work.popleft()
            fn()


_NC_CACHE = None


def kernel(query, key, value, mask, wq, bq, wk, bk, wv, bv, wo, bo):
    global _NC_CACHE
    if _NC_CACHE is None:
        _NC_CACHE = _build()
    nc = _NC_CACHE

    query = np.asarray(query, dtype=np.float32)
    key = np.asarray(key, dtype=np.float32)
    value = np.asarray(value, dtype=np.float32)
    mask = np.asarray(mask, dtype=np.float32)
    wq_np = np.asarray(wq, np.float32)
    wk_np = np.asarray(wk, np.float32)
    wv_np = np.asarray(wv, np.float32)
    wo_np = np.asarray(wo, np.float32)
    bq_np = np.asarray(bq, np.float32)
    bk_np = np.asarray(bk, np.float32)
    bias_out = (np.asarray(bo, np.float64) +
                np.asarray(bv, np.float64) @ np.asarray(wo_np, np.float64)
                ).astype(np.float32)

    xT = {}
    for b in range(B):
        xT[b] = (np.ascontiguousarray(query[b].T).astype(np_bf16),
                 np.ascontiguousarray(key[b].T).astype(np_bf16),
                 np.ascontiguousarray(value[b].T).astype(np_bf16))
    shared_g = []
    for g in range(2):
        cols = slice(DG * g, DG * (g + 1))
        bqk_host = np.zeros((128, 8), np.float32)
        for cc in range(4):
            bqk_host[:, cc] = bq_np[cols][cc * 128:(cc + 1) * 128]
            bqk_host[:, 4 + cc] = bk_np[cols][cc * 128:(cc + 1) * 128]
        shared_g.append({
            "wq": np.ascontiguousarray(wq_np[:, cols]).astype(np_bf16),
            "wk": np.ascontiguousarray(wk_np[:, cols]).astype(np_bf16),
            "wv": np.ascontiguousarray(wv_np[:, cols]).astype(np_bf16),
            "wo": np.ascontiguousarray(wo_np[cols, :]).astype(np_bf16),
            "bqk": bqk_host,
        })
    ones_host = np.ones((128, 256), np_bf16)

    in_maps = []
    for core in range(N_CORES):
        b, g = divmod(core, 2)
        em = np.exp(mask[b, 0, 0] * np.float32(-1e9)).astype(np.float32)
        emc = np.ascontiguousarray(em.reshape(SKC, 128).T)   # [128, SKC]
        in_maps.append({
            "xq": xT[b][0], "xk": xT[b][1], "xv": xT[b][2],
            "emf": emc, "ones2": ones_host,
            **shared_g[g],
        })

    res = run_bass_kernel_spmd(nc, in_maps, core_ids=list(range(N_CORES)))
    full = np.empty((B, S, D), np.float32)
    for b in range(B):
        full[b] = res.results[2 * b]["out"]
        full[b] += res.results[2 * b + 1]["out"]
        full[b] += bias_out
    return full
